# revision 12
# baseline (speedup 1.0000x reference)
"""Trainium2 Bass kernel for nn_CustomSegmentationLayer (retrieval_knn).

Pipeline per image (reference semantics):
  - sample 50 fg + 50 bg training pixels via jax RNG (host, bit-exact
    vmapped replication of the reference's sampling)
  - 5-D pixel features (3 color + 2 weighted position), standardized by
    train mean/std
  - brute-force 5-NN over the 100 train pixels, fg-vote >= 2/5 -> seg mask
  - output = preprocessed image masked by seg

Device formulation: for pixel p and train candidate j,
    m[p, j] = 2*t_p.s_j - ||s_j||^2   (t = standardized test feat,
                                       s = standardized train feat)
is a monotone (reversed) proxy for squared distance, computed as one
K=6 matmul per 128-pixel tile:  feats[6,128].T @ W[6,100], where the
host folds standardization, the 2x, the /255^2 color scale and the
||s||^2 bias (via a ones-row) into W.

Vote rule: seg_p = 1  iff  2nd-smallest fg distance <= 4th-smallest bg
distance, i.e. fgmax8[:,1] >= bgmax8[:,3] on m (ties favor fg exactly
like the reference's index-ordered top_k). One vector.max (top-8 per
partition) per 50-candidate half per tile.

Sharding: pure data parallel, 4 images per core on 8 cores.
"""

import numpy as np

H = W = 96
C = 3
N_PIX = H * W          # 9216
NPC = 50               # train samples per class
N_CAND = 2 * NPC       # 100
B = 32                 # batch
N_CORES = 8
IPC = B // N_CORES     # images per core = 4
NT = N_PIX // 128      # 72 pixel tiles per image
GROUP = 5              # score tiles per PSUM bank (5*100 <= 512)
POS_WEIGHT = 100.0
COLOR_SCALE = 255.0 * 255.0  # raw -> feature scale for color dims

_CACHE = {}


def _host_sampling(images_np):
    """Replicate the reference's vmapped sampling + standardization on CPU.

    Must mirror the reference *under vmap* — jax's batched RNG produces
    different streams than per-image calls.
    Returns train_s [B,100,5], mean [B,5], std [B,5] (fp32, bit-exact
    w.r.t. the reference).
    """
    import jax
    import jax.numpy as jnp
    from jax import lax

    cpu = jax.devices("cpu")[0]

    def sample_part(img, key):
        img_p = jnp.clip(img / 255.0, 0.0, 1.0)
        img_norm = img_p / jnp.max(img_p, axis=(0, 1), keepdims=True)
        cleaned = img_norm * (img_norm < 0.6).astype(jnp.float32)
        fg = jnp.any(cleaned > 0, axis=-1).reshape(-1)
        bg = ~fg

        k_fg, k_bg = jax.random.split(key)

        def samp(k, mask):
            u = jax.random.uniform(k, (N_PIX,))
            score = jnp.where(mask, u, -1.0)
            _, idx = lax.top_k(score, NPC)
            return idx

        fg_idx = samp(k_fg, fg)
        bg_idx = samp(k_bg, bg)

        ii, jj = jnp.meshgrid(jnp.arange(H), jnp.arange(W), indexing="ij")
        pos = jnp.stack([ii, jj], axis=-1).reshape(-1, 2).astype(jnp.float32)
        pos = pos / jnp.array([H, W], jnp.float32) * POS_WEIGHT
        feats_all = jnp.concatenate([img_p.reshape(-1, C) / 255.0, pos], axis=1)
        train = jnp.concatenate([feats_all[fg_idx], feats_all[bg_idx]], axis=0)
        mean = jnp.mean(train, axis=0)
        std = jnp.std(train, axis=0)
        train_s = (train - mean) / std
        return train_s, mean, std

    with jax.default_device(cpu):
        keys = jax.random.split(jax.random.key(42), B)
        train_s, mean, std = jax.vmap(sample_part)(jnp.asarray(images_np), keys)
        return (np.asarray(train_s), np.asarray(mean), np.asarray(std))


def _build_weights(train_s, mean, std):
    """Fold standardization + bias into W [B, 6, 100] fp32 (f64 math)."""
    s = train_s.astype(np.float64)          # [B,100,5]
    mn = mean.astype(np.float64)            # [B,5]
    sd = std.astype(np.float64)             # [B,5]
    a = s / sd[:, None, :]                  # [B,100,5]
    Wm = np.empty((B, 6, N_CAND), np.float64)
    Wm[:, 0:3, :] = (2.0 * a[:, :, 0:3] / COLOR_SCALE).transpose(0, 2, 1)
    Wm[:, 3:5, :] = (2.0 * a[:, :, 3:5]).transpose(0, 2, 1)
    Wm[:, 5, :] = -np.sum(s * s, axis=2) - 2.0 * np.sum(mn[:, None, :] * a, axis=2)
    return Wm.astype(np.float32)


def _pos_features():
    ii, jj = np.meshgrid(np.arange(H), np.arange(W), indexing="ij")
    pos = np.stack([ii, jj], -1).reshape(-1, 2).astype(np.float32)
    return pos / np.array([H, W], np.float32) * np.float32(POS_WEIGHT)  # [N_PIX,2]


def _build_bass():
    import concourse.bacc as bacc
    import concourse.mybir as mybir
    from concourse import tile

    f32 = mybir.dt.float32
    f16 = mybir.dt.float16
    nc = bacc.Bacc("TRN2", target_bir_lowering=False, debug=False)

    # fp16 hi/lo split of features and weights: scores computed as
    # fh.W_hi + fh.W_lo + fl.W_hi (fp32 PSUM accum); residual term
    # fl.W_lo ~1e-5 << the 1.2e-4 min decision margin.
    fh_d = nc.dram_tensor("feats_hi", [IPC, 6, N_PIX], f16, kind="ExternalInput")
    fl_d = nc.dram_tensor("feats_lo", [IPC, 6, N_PIX], f16, kind="ExternalInput")
    wh_d = nc.dram_tensor("w_hi", [6, IPC, N_CAND], f16, kind="ExternalInput")
    wl_d = nc.dram_tensor("w_lo", [6, IPC, N_CAND], f16, kind="ExternalInput")
    imgpm_d = nc.dram_tensor("imgpm", [IPC, 128, NT * C], f32, kind="ExternalInput")
    out_d = nc.dram_tensor("out", [IPC, 128, NT * C], f32, kind="ExternalOutput")

    # 4-way row-group packing: feats/weights replicated at partition bases
    # 0/32/64/96 so four K=6 matmuls run concurrently in distinct 32-row
    # strips of the PE array (tile_position), each writing its own PSUM bank.
    NROW = 4
    QR = 4                      # rr-rounds per quad-round (psum offsets)
    TILES_PER_QR = NROW * QR    # 16
    n_qr = (NT + TILES_PER_QR - 1) // TILES_PER_QR  # 5 (last partial: 8 tiles)

    with tile.TileContext(nc) as tc:
        with (
            tc.tile_pool(name="const", bufs=1) as cpool,
            tc.tile_pool(name="sb", bufs=2) as sb,
            tc.tile_pool(name="scores", bufs=2) as scp,
            tc.tile_pool(name="psum", bufs=2, space="PSUM") as pp,
        ):
            wh_sb = cpool.tile([128, IPC, N_CAND], f16)
            wl_sb = cpool.tile([128, IPC, N_CAND], f16)
            for j in range(NROW):
                nc.sync.dma_start(out=wh_sb[32 * j : 32 * j + 6], in_=wh_d[:])
                nc.sync.dma_start(out=wl_sb[32 * j : 32 * j + 6], in_=wl_d[:])

            for i in range(IPC):
                fh_sb = sb.tile([128, N_PIX], f16, tag="fh")
                fl_sb = sb.tile([128, N_PIX], f16, tag="fl")
                for j in range(NROW):
                    nc.sync.dma_start(out=fh_sb[32 * j : 32 * j + 6], in_=fh_d[i])
                    nc.sync.dma_start(out=fl_sb[32 * j : 32 * j + 6], in_=fl_d[i])
                imgpm_sb = sb.tile([128, NT * C], f32, tag="imgpm")
                nc.sync.dma_start(out=imgpm_sb[:], in_=imgpm_d[i])

                fgmax = sb.tile([128, NT * 8], f32, tag="fgmax")
                bgmax = sb.tile([128, NT * 8], f32, tag="bgmax")

                for R in range(n_qr):
                    nrr = min(QR, (NT - R * TILES_PER_QR) // NROW)
                    ps = [
                        pp.tile([128, QR * N_CAND], f32, tag=f"ps{j}", name=f"ps{j}")
                        for j in range(NROW)
                    ]
                    # interleave row groups for PE concurrency
                    for rr in range(nrr):
                        for term in range(3):
                            for j in range(NROW):
                                t = R * TILES_PER_QR + rr * NROW + j
                                pslice = ps[j][:, rr * N_CAND : (rr + 1) * N_CAND]
                                rbase = 32 * j
                                fh_t = fh_sb[rbase : rbase + 6, t * 128 : (t + 1) * 128]
                                fl_t = fl_sb[rbase : rbase + 6, t * 128 : (t + 1) * 128]
                                if term == 0:
                                    nc.tensor.matmul(
                                        pslice, fh_t, wh_sb[rbase : rbase + 6, i, :],
                                        start=True, stop=False,
                                        tile_position=(rbase, 0),
                                    )
                                elif term == 1:
                                    nc.tensor.matmul(
                                        pslice, fh_t, wl_sb[rbase : rbase + 6, i, :],
                                        start=False, stop=False,
                                        tile_position=(rbase, 0),
                                    )
                                else:
                                    nc.tensor.matmul(
                                        pslice, fl_t, wh_sb[rbase : rbase + 6, i, :],
                                        start=False, stop=True,
                                        tile_position=(rbase, 0),
                                    )
                    for j in range(NROW):
                        sc = scp.tile([128, QR * N_CAND], f32, tag=f"sc{j}")
                        nc.scalar.copy(
                            out=sc[:, : nrr * N_CAND], in_=ps[j][:, : nrr * N_CAND]
                        )
                        for rr in range(nrr):
                            t = R * TILES_PER_QR + rr * NROW + j
                            nc.vector.max(
                                out=fgmax[:, t * 8 : (t + 1) * 8],
                                in_=sc[:, rr * N_CAND : rr * N_CAND + NPC],
                            )
                            nc.vector.max(
                                out=bgmax[:, t * 8 : (t + 1) * 8],
                                in_=sc[:, rr * N_CAND + NPC : (rr + 1) * N_CAND],
                            )

                # seg = (2nd-largest fg m >= 4th-largest bg m) scaled by 1/255
                seg = sb.tile([128, NT], f32, tag="seg")
                fgmax_r = fgmax[:].rearrange("p (t k) -> p t k", k=8)
                bgmax_r = bgmax[:].rearrange("p (t k) -> p t k", k=8)
                nc.vector.tensor_tensor(
                    seg[:], fgmax_r[:, :, 1], bgmax_r[:, :, 3],
                    mybir.AluOpType.is_ge,
                )
                nc.vector.tensor_scalar_mul(seg[:], seg[:], 1.0 / 255.0)

                out_sb = sb.tile([128, NT * C], f32, tag="out")
                img_r = imgpm_sb[:].rearrange("p (t c) -> p t c", c=C)
                out_r = out_sb[:].rearrange("p (t c) -> p t c", c=C)
                nc.vector.tensor_tensor(
                    out_r, img_r,
                    seg[:, :, None].to_broadcast([128, NT, C]),
                    mybir.AluOpType.mult,
                )
                nc.sync.dma_start(out=out_d[i], in_=out_sb[:])

    nc.compile()
    return nc


def _get_nc():
    if "nc" not in _CACHE:
        _CACHE["nc"] = _build_bass()
    return _CACHE["nc"]


def prepare_in_maps(images: np.ndarray) -> list:
    """Host preamble: sampling + weight folding + device data layouts."""
    images = np.asarray(images, dtype=np.float32)
    assert images.shape == (B, H, W, C)

    train_s, mean, std = _host_sampling(images)
    Wall = _build_weights(train_s, mean, std)        # [B,6,100]
    pos = _pos_features()                            # [N_PIX,2]

    flat = images.reshape(B, N_PIX, C)
    feats = np.empty((B, 6, N_PIX), np.float32)
    feats[:, 0:3, :] = flat.transpose(0, 2, 1)
    feats[:, 3:5, :] = pos.T[None]
    feats[:, 5, :] = 1.0
    f16h = feats.astype(np.float16)
    f16l = (feats - f16h.astype(np.float32)).astype(np.float16)
    w16h = Wall.astype(np.float16)
    w16l = (Wall - w16h.astype(np.float32)).astype(np.float16)
    # pixel-major tiles: imgpm[b, p, t*3+c] = img[b, t*128+p, c]
    imgpm = np.ascontiguousarray(
        flat.reshape(B, NT, 128, C).transpose(0, 2, 1, 3)
    ).reshape(B, 128, NT * C)

    in_maps = []
    for c in range(N_CORES):
        sl = slice(c * IPC, (c + 1) * IPC)
        in_maps.append(
            {
                "feats_hi": np.ascontiguousarray(f16h[sl]),
                "feats_lo": np.ascontiguousarray(f16l[sl]),
                "w_hi": np.ascontiguousarray(w16h[sl].transpose(1, 0, 2)),
                "w_lo": np.ascontiguousarray(w16l[sl].transpose(1, 0, 2)),
                "imgpm": np.ascontiguousarray(imgpm[sl]),
            }
        )
    return in_maps


def assemble_output(results: list) -> np.ndarray:
    out = np.empty((B, N_PIX, C), np.float32)
    for c in range(N_CORES):
        o = results[c]["out"]  # [IPC, 128, NT*C]
        o = o.reshape(IPC, 128, NT, C).transpose(0, 2, 1, 3).reshape(IPC, N_PIX, C)
        out[c * IPC : (c + 1) * IPC] = o
    return out.reshape(B, H, W, C)


def kernel(images: np.ndarray) -> np.ndarray:
    from concourse.bass_utils import run_bass_kernel_spmd

    in_maps = prepare_in_maps(images)
    nc = _get_nc()
    res = run_bass_kernel_spmd(nc, in_maps, core_ids=list(range(N_CORES)))
    return assemble_output(res.results)


# revision 14
# speedup vs baseline: 1.4489x; 1.4489x over previous
"""Trainium2 Bass kernel for nn_CustomSegmentationLayer (retrieval_knn).

Pipeline per image (reference semantics):
  - sample 50 fg + 50 bg training pixels via jax RNG (host, bit-exact
    vmapped replication of the reference's sampling)
  - 5-D pixel features (3 color + 2 weighted position), standardized by
    train mean/std
  - brute-force 5-NN over the 100 train pixels, fg-vote >= 2/5 -> seg mask
  - output = preprocessed image masked by seg

Device formulation: for pixel p and train candidate j,
    m[p, j] = 2*t_p.s_j - ||s_j||^2   (t = standardized test feat,
                                       s = standardized train feat)
is a monotone (reversed) proxy for squared distance, computed as one
K=6 matmul per 128-pixel tile:  feats[6,128].T @ W[6,100], where the
host folds standardization, the 2x, the /255^2 color scale and the
||s||^2 bias (via a ones-row) into W.

Vote rule: seg_p = 1  iff  2nd-smallest fg distance <= 4th-smallest bg
distance, i.e. fgmax8[:,1] >= bgmax8[:,3] on m (ties favor fg exactly
like the reference's index-ordered top_k). One vector.max (top-8 per
partition) per 50-candidate half per tile.

Sharding: pure data parallel, 4 images per core on 8 cores.
"""

import numpy as np

H = W = 96
C = 3
N_PIX = H * W          # 9216
NPC = 50               # train samples per class
N_CAND = 2 * NPC       # 100
B = 32                 # batch
N_CORES = 8
IPC = B // N_CORES     # images per core = 4
NT = N_PIX // 128      # 72 pixel tiles per image
GROUP = 5              # score tiles per PSUM bank (5*100 <= 512)
POS_WEIGHT = 100.0
COLOR_SCALE = 255.0 * 255.0  # raw -> feature scale for color dims

_CACHE = {}


def _host_sampling(images_np):
    """Replicate the reference's vmapped sampling + standardization on CPU.

    Must mirror the reference *under vmap* — jax's batched RNG produces
    different streams than per-image calls.
    Returns train_s [B,100,5], mean [B,5], std [B,5] (fp32, bit-exact
    w.r.t. the reference).
    """
    import jax
    import jax.numpy as jnp
    from jax import lax

    cpu = jax.devices("cpu")[0]

    def sample_part(img, key):
        img_p = jnp.clip(img / 255.0, 0.0, 1.0)
        img_norm = img_p / jnp.max(img_p, axis=(0, 1), keepdims=True)
        cleaned = img_norm * (img_norm < 0.6).astype(jnp.float32)
        fg = jnp.any(cleaned > 0, axis=-1).reshape(-1)
        bg = ~fg

        k_fg, k_bg = jax.random.split(key)

        def samp(k, mask):
            u = jax.random.uniform(k, (N_PIX,))
            score = jnp.where(mask, u, -1.0)
            _, idx = lax.top_k(score, NPC)
            return idx

        fg_idx = samp(k_fg, fg)
        bg_idx = samp(k_bg, bg)

        ii, jj = jnp.meshgrid(jnp.arange(H), jnp.arange(W), indexing="ij")
        pos = jnp.stack([ii, jj], axis=-1).reshape(-1, 2).astype(jnp.float32)
        pos = pos / jnp.array([H, W], jnp.float32) * POS_WEIGHT
        feats_all = jnp.concatenate([img_p.reshape(-1, C) / 255.0, pos], axis=1)
        train = jnp.concatenate([feats_all[fg_idx], feats_all[bg_idx]], axis=0)
        mean = jnp.mean(train, axis=0)
        std = jnp.std(train, axis=0)
        train_s = (train - mean) / std
        return train_s, mean, std

    with jax.default_device(cpu):
        keys = jax.random.split(jax.random.key(42), B)
        train_s, mean, std = jax.vmap(sample_part)(jnp.asarray(images_np), keys)
        return (np.asarray(train_s), np.asarray(mean), np.asarray(std))


def _build_weights(train_s, mean, std):
    """Fold standardization + bias into W [B, 6, 100] fp32 (f64 math)."""
    s = train_s.astype(np.float64)          # [B,100,5]
    mn = mean.astype(np.float64)            # [B,5]
    sd = std.astype(np.float64)             # [B,5]
    a = s / sd[:, None, :]                  # [B,100,5]
    Wm = np.empty((B, 6, N_CAND), np.float64)
    Wm[:, 0:3, :] = (2.0 * a[:, :, 0:3] / COLOR_SCALE).transpose(0, 2, 1)
    Wm[:, 3:5, :] = (2.0 * a[:, :, 3:5]).transpose(0, 2, 1)
    Wm[:, 5, :] = -np.sum(s * s, axis=2) - 2.0 * np.sum(mn[:, None, :] * a, axis=2)
    return Wm.astype(np.float32)


def _pos_features():
    ii, jj = np.meshgrid(np.arange(H), np.arange(W), indexing="ij")
    pos = np.stack([ii, jj], -1).reshape(-1, 2).astype(np.float32)
    return pos / np.array([H, W], np.float32) * np.float32(POS_WEIGHT)  # [N_PIX,2]


def _build_bass():
    import concourse.bacc as bacc
    import concourse.mybir as mybir
    from concourse import tile

    f32 = mybir.dt.float32
    f16 = mybir.dt.float16
    nc = bacc.Bacc("TRN2", target_bir_lowering=False, debug=False)

    # fp16 hi/lo split folded into ONE K=24 matmul per tile:
    #   lhsT rows = [fh(6); fl(6); fh(6); fl(6)]
    #   rhs  rows = [Wh(6); Wh(6); Wl(6); Wl(6)]
    # so one MM accumulates fh.Wh + fl.Wh + fh.Wl + fl.Wl — the full
    # fp32-accurate product. MM time is N cycles regardless of K.
    f24_d = nc.dram_tensor("feats24", [IPC, 24, N_PIX], f16, kind="ExternalInput")
    w24_d = nc.dram_tensor("w24", [24, IPC, N_CAND], f16, kind="ExternalInput")
    imgpm_d = nc.dram_tensor("imgpm", [IPC, 128, NT * C], f32, kind="ExternalInput")
    out_d = nc.dram_tensor("out", [IPC, 128, NT * C], f32, kind="ExternalOutput")

    n_groups = (NT + GROUP - 1) // GROUP

    with tile.TileContext(nc) as tc:
        with (
            tc.tile_pool(name="const", bufs=1) as cpool,
            tc.tile_pool(name="sb", bufs=2) as sb,
            tc.tile_pool(name="scores", bufs=3) as scp,
            tc.tile_pool(name="psum", bufs=6, space="PSUM") as pp,
        ):
            w24_sb = cpool.tile([24, IPC, N_CAND], f16)
            nc.sync.dma_start(out=w24_sb[:], in_=w24_d[:])

            for i in range(IPC):
                f24_sb = sb.tile([24, N_PIX], f16, tag="f24")
                nc.sync.dma_start(out=f24_sb[:], in_=f24_d[i])
                imgpm_sb = sb.tile([128, NT * C], f32, tag="imgpm")
                nc.sync.dma_start(out=imgpm_sb[:], in_=imgpm_d[i])

                fgmax = sb.tile([128, NT * 8], f32, tag="fgmax")
                bgmax = sb.tile([128, NT * 8], f32, tag="bgmax")

                for g in range(n_groups):
                    ntg = min(GROUP, NT - g * GROUP)
                    ps = pp.tile([128, GROUP * N_CAND], f32, tag="ps")
                    for q in range(ntg):
                        t = g * GROUP + q
                        nc.tensor.matmul(
                            ps[:, q * N_CAND : (q + 1) * N_CAND],
                            f24_sb[:, t * 128 : (t + 1) * 128],
                            w24_sb[:, i, :],
                            start=True,
                            stop=True,
                        )
                    sc = scp.tile([128, GROUP * N_CAND], f32, tag="sc")
                    nc.scalar.copy(
                        out=sc[:, : ntg * N_CAND], in_=ps[:, : ntg * N_CAND]
                    )
                    for q in range(ntg):
                        t = g * GROUP + q
                        nc.vector.max(
                            out=fgmax[:, t * 8 : (t + 1) * 8],
                            in_=sc[:, q * N_CAND : q * N_CAND + NPC],
                        )
                        nc.vector.max(
                            out=bgmax[:, t * 8 : (t + 1) * 8],
                            in_=sc[:, q * N_CAND + NPC : (q + 1) * N_CAND],
                        )

                # seg = (2nd-largest fg m >= 4th-largest bg m) scaled by 1/255
                seg = sb.tile([128, NT], f32, tag="seg")
                fgmax_r = fgmax[:].rearrange("p (t k) -> p t k", k=8)
                bgmax_r = bgmax[:].rearrange("p (t k) -> p t k", k=8)
                nc.vector.tensor_tensor(
                    seg[:], fgmax_r[:, :, 1], bgmax_r[:, :, 3],
                    mybir.AluOpType.is_ge,
                )
                nc.vector.tensor_scalar_mul(seg[:], seg[:], 1.0 / 255.0)

                out_sb = sb.tile([128, NT * C], f32, tag="out")
                img_r = imgpm_sb[:].rearrange("p (t c) -> p t c", c=C)
                out_r = out_sb[:].rearrange("p (t c) -> p t c", c=C)
                nc.vector.tensor_tensor(
                    out_r, img_r,
                    seg[:, :, None].to_broadcast([128, NT, C]),
                    mybir.AluOpType.mult,
                )
                nc.sync.dma_start(out=out_d[i], in_=out_sb[:])

    nc.compile()
    return nc


def _get_nc():
    if "nc" not in _CACHE:
        _CACHE["nc"] = _build_bass()
    return _CACHE["nc"]


def prepare_in_maps(images: np.ndarray) -> list:
    """Host preamble: sampling + weight folding + device data layouts."""
    images = np.asarray(images, dtype=np.float32)
    assert images.shape == (B, H, W, C)

    train_s, mean, std = _host_sampling(images)
    Wall = _build_weights(train_s, mean, std)        # [B,6,100]
    pos = _pos_features()                            # [N_PIX,2]

    flat = images.reshape(B, N_PIX, C)
    feats = np.empty((B, 6, N_PIX), np.float32)
    feats[:, 0:3, :] = flat.transpose(0, 2, 1)
    feats[:, 3:5, :] = pos.T[None]
    feats[:, 5, :] = 1.0
    f16h = feats.astype(np.float16)
    f16l = (feats - f16h.astype(np.float32)).astype(np.float16)
    w16h = Wall.astype(np.float16)
    w16l = (Wall - w16h.astype(np.float32)).astype(np.float16)
    # K=24 stacking: feats rows [fh; fl; fh; fl], W rows [Wh; Wh; Wl; Wl]
    f24 = np.concatenate([f16h, f16l, f16h, f16l], axis=1)          # [B,24,NPIX]
    w24 = np.concatenate([w16h, w16h, w16l, w16l], axis=1)          # [B,24,100]
    # pixel-major tiles: imgpm[b, p, t*3+c] = img[b, t*128+p, c]
    imgpm = np.ascontiguousarray(
        flat.reshape(B, NT, 128, C).transpose(0, 2, 1, 3)
    ).reshape(B, 128, NT * C)

    in_maps = []
    for c in range(N_CORES):
        sl = slice(c * IPC, (c + 1) * IPC)
        in_maps.append(
            {
                "feats24": np.ascontiguousarray(f24[sl]),
                "w24": np.ascontiguousarray(w24[sl].transpose(1, 0, 2)),
                "imgpm": np.ascontiguousarray(imgpm[sl]),
            }
        )
    return in_maps


def assemble_output(results: list) -> np.ndarray:
    out = np.empty((B, N_PIX, C), np.float32)
    for c in range(N_CORES):
        o = results[c]["out"]  # [IPC, 128, NT*C]
        o = o.reshape(IPC, 128, NT, C).transpose(0, 2, 1, 3).reshape(IPC, N_PIX, C)
        out[c * IPC : (c + 1) * IPC] = o
    return out.reshape(B, H, W, C)


def kernel(images: np.ndarray) -> np.ndarray:
    from concourse.bass_utils import run_bass_kernel_spmd

    in_maps = prepare_in_maps(images)
    nc = _get_nc()
    res = run_bass_kernel_spmd(nc, in_maps, core_ids=list(range(N_CORES)))
    return assemble_output(res.results)


# revision 15
# speedup vs baseline: 1.4896x; 1.0281x over previous
"""Trainium2 Bass kernel for nn_CustomSegmentationLayer (retrieval_knn).

Pipeline per image (reference semantics):
  - sample 50 fg + 50 bg training pixels via jax RNG (host, bit-exact
    vmapped replication of the reference's sampling)
  - 5-D pixel features (3 color + 2 weighted position), standardized by
    train mean/std
  - brute-force 5-NN over the 100 train pixels, fg-vote >= 2/5 -> seg mask
  - output = preprocessed image masked by seg

Device formulation: for pixel p and train candidate j,
    m[p, j] = 2*t_p.s_j - ||s_j||^2   (t = standardized test feat,
                                       s = standardized train feat)
is a monotone (reversed) proxy for squared distance, computed as one
K=6 matmul per 128-pixel tile:  feats[6,128].T @ W[6,100], where the
host folds standardization, the 2x, the /255^2 color scale and the
||s||^2 bias (via a ones-row) into W.

Vote rule: seg_p = 1  iff  2nd-smallest fg distance <= 4th-smallest bg
distance, i.e. fgmax8[:,1] >= bgmax8[:,3] on m (ties favor fg exactly
like the reference's index-ordered top_k). One vector.max (top-8 per
partition) per 50-candidate half per tile.

Sharding: pure data parallel, 4 images per core on 8 cores.
"""

import numpy as np

H = W = 96
C = 3
N_PIX = H * W          # 9216
NPC = 50               # train samples per class
N_CAND = 2 * NPC       # 100
B = 32                 # batch
N_CORES = 8
IPC = B // N_CORES     # images per core = 4
NT = N_PIX // 128      # 72 pixel tiles per image
GROUP = 5              # score tiles per PSUM bank (5*100 <= 512)
POS_WEIGHT = 100.0
COLOR_SCALE = 255.0 * 255.0  # raw -> feature scale for color dims

_CACHE = {}


def _host_sampling(images_np):
    """Replicate the reference's vmapped sampling + standardization on CPU.

    Must mirror the reference *under vmap* — jax's batched RNG produces
    different streams than per-image calls.
    Returns train_s [B,100,5], mean [B,5], std [B,5] (fp32, bit-exact
    w.r.t. the reference).
    """
    import jax
    import jax.numpy as jnp
    from jax import lax

    cpu = jax.devices("cpu")[0]

    def sample_part(img, key):
        img_p = jnp.clip(img / 255.0, 0.0, 1.0)
        img_norm = img_p / jnp.max(img_p, axis=(0, 1), keepdims=True)
        cleaned = img_norm * (img_norm < 0.6).astype(jnp.float32)
        fg = jnp.any(cleaned > 0, axis=-1).reshape(-1)
        bg = ~fg

        k_fg, k_bg = jax.random.split(key)

        def samp(k, mask):
            u = jax.random.uniform(k, (N_PIX,))
            score = jnp.where(mask, u, -1.0)
            _, idx = lax.top_k(score, NPC)
            return idx

        fg_idx = samp(k_fg, fg)
        bg_idx = samp(k_bg, bg)

        ii, jj = jnp.meshgrid(jnp.arange(H), jnp.arange(W), indexing="ij")
        pos = jnp.stack([ii, jj], axis=-1).reshape(-1, 2).astype(jnp.float32)
        pos = pos / jnp.array([H, W], jnp.float32) * POS_WEIGHT
        feats_all = jnp.concatenate([img_p.reshape(-1, C) / 255.0, pos], axis=1)
        train = jnp.concatenate([feats_all[fg_idx], feats_all[bg_idx]], axis=0)
        mean = jnp.mean(train, axis=0)
        std = jnp.std(train, axis=0)
        train_s = (train - mean) / std
        return train_s, mean, std

    with jax.default_device(cpu):
        keys = jax.random.split(jax.random.key(42), B)
        train_s, mean, std = jax.vmap(sample_part)(jnp.asarray(images_np), keys)
        return (np.asarray(train_s), np.asarray(mean), np.asarray(std))


def _build_weights(train_s, mean, std):
    """Fold standardization + bias into W [B, 6, 100] fp32 (f64 math)."""
    s = train_s.astype(np.float64)          # [B,100,5]
    mn = mean.astype(np.float64)            # [B,5]
    sd = std.astype(np.float64)             # [B,5]
    a = s / sd[:, None, :]                  # [B,100,5]
    Wm = np.empty((B, 6, N_CAND), np.float64)
    Wm[:, 0:3, :] = (2.0 * a[:, :, 0:3] / COLOR_SCALE).transpose(0, 2, 1)
    Wm[:, 3:5, :] = (2.0 * a[:, :, 3:5]).transpose(0, 2, 1)
    Wm[:, 5, :] = -np.sum(s * s, axis=2) - 2.0 * np.sum(mn[:, None, :] * a, axis=2)
    return Wm.astype(np.float32)


def _pos_features():
    ii, jj = np.meshgrid(np.arange(H), np.arange(W), indexing="ij")
    pos = np.stack([ii, jj], -1).reshape(-1, 2).astype(np.float32)
    return pos / np.array([H, W], np.float32) * np.float32(POS_WEIGHT)  # [N_PIX,2]


def _build_bass():
    import concourse.bacc as bacc
    import concourse.mybir as mybir
    from concourse import tile

    f32 = mybir.dt.float32
    f16 = mybir.dt.float16
    nc = bacc.Bacc("TRN2", target_bir_lowering=False, debug=False)

    # fp16 hi/lo split folded into ONE K=24 matmul per tile:
    #   lhsT rows = [fh(6); fl(6); fh(6); fl(6)]
    #   rhs  rows = [Wh(6); Wh(6); Wl(6); Wl(6)]
    # so one MM accumulates fh.Wh + fl.Wh + fh.Wl + fl.Wl — the full
    # fp32-accurate product. MM time is N cycles regardless of K.
    f24_d = nc.dram_tensor("feats24", [IPC, 24, N_PIX], f16, kind="ExternalInput")
    w24_d = nc.dram_tensor("w24", [24, IPC, N_CAND], f16, kind="ExternalInput")
    imgpm_d = nc.dram_tensor("imgpm", [IPC, 128, NT * C], f32, kind="ExternalInput")
    out_d = nc.dram_tensor("out", [IPC, 128, NT * C], f32, kind="ExternalOutput")

    n_groups = (NT + GROUP - 1) // GROUP

    with tile.TileContext(nc) as tc:
        with (
            tc.tile_pool(name="const", bufs=1) as cpool,
            tc.tile_pool(name="sb", bufs=2) as sb,
            tc.tile_pool(name="scores", bufs=3) as scp,
            tc.tile_pool(name="psum", bufs=6, space="PSUM") as pp,
        ):
            w24_sb = cpool.tile([24, IPC, N_CAND], f16)
            nc.sync.dma_start(out=w24_sb[:], in_=w24_d[:])

            # feats arrive in 3 chunk-tiles (5 groups each) so group 0's
            # matmuls start after ~1/3 of the image's features land.
            CHUNK_GROUPS = 5
            CHUNK_PX = CHUNK_GROUPS * GROUP * 128  # 3200
            for i in range(IPC):
                f24_ch = []
                for ci in range(3):
                    px0 = ci * CHUNK_PX
                    px1 = min(N_PIX, (ci + 1) * CHUNK_PX)
                    ch = sb.tile([24, px1 - px0], f16, tag=f"f24_{ci}", name=f"f24_{ci}")
                    nc.sync.dma_start(out=ch[:], in_=f24_d[i, :, px0:px1])
                    f24_ch.append(ch)
                imgpm_sb = sb.tile([128, NT * C], f32, tag="imgpm")
                nc.sync.dma_start(out=imgpm_sb[:], in_=imgpm_d[i])

                fgmax = sb.tile([128, NT * 8], f32, tag="fgmax")
                bgmax = sb.tile([128, NT * 8], f32, tag="bgmax")

                for g in range(n_groups):
                    ntg = min(GROUP, NT - g * GROUP)
                    ps = pp.tile([128, GROUP * N_CAND], f32, tag="ps")
                    for q in range(ntg):
                        t = g * GROUP + q
                        ci = (t * 128) // CHUNK_PX
                        off = t * 128 - ci * CHUNK_PX
                        nc.tensor.matmul(
                            ps[:, q * N_CAND : (q + 1) * N_CAND],
                            f24_ch[ci][:, off : off + 128],
                            w24_sb[:, i, :],
                            start=True,
                            stop=True,
                        )
                    sc = scp.tile([128, GROUP * N_CAND], f32, tag="sc")
                    nc.scalar.copy(
                        out=sc[:, : ntg * N_CAND], in_=ps[:, : ntg * N_CAND]
                    )
                    for q in range(ntg):
                        t = g * GROUP + q
                        nc.vector.max(
                            out=fgmax[:, t * 8 : (t + 1) * 8],
                            in_=sc[:, q * N_CAND : q * N_CAND + NPC],
                        )
                        nc.vector.max(
                            out=bgmax[:, t * 8 : (t + 1) * 8],
                            in_=sc[:, q * N_CAND + NPC : (q + 1) * N_CAND],
                        )

                # seg = (2nd-largest fg m >= 4th-largest bg m) scaled by 1/255
                seg = sb.tile([128, NT], f32, tag="seg")
                fgmax_r = fgmax[:].rearrange("p (t k) -> p t k", k=8)
                bgmax_r = bgmax[:].rearrange("p (t k) -> p t k", k=8)
                nc.vector.tensor_tensor(
                    seg[:], fgmax_r[:, :, 1], bgmax_r[:, :, 3],
                    mybir.AluOpType.is_ge,
                )
                nc.vector.tensor_scalar_mul(seg[:], seg[:], 1.0 / 255.0)

                out_sb = sb.tile([128, NT * C], f32, tag="out")
                img_r = imgpm_sb[:].rearrange("p (t c) -> p t c", c=C)
                out_r = out_sb[:].rearrange("p (t c) -> p t c", c=C)
                nc.vector.tensor_tensor(
                    out_r, img_r,
                    seg[:, :, None].to_broadcast([128, NT, C]),
                    mybir.AluOpType.mult,
                )
                nc.sync.dma_start(out=out_d[i], in_=out_sb[:])

    nc.compile()
    return nc


def _get_nc():
    if "nc" not in _CACHE:
        _CACHE["nc"] = _build_bass()
    return _CACHE["nc"]


def prepare_in_maps(images: np.ndarray) -> list:
    """Host preamble: sampling + weight folding + device data layouts."""
    images = np.asarray(images, dtype=np.float32)
    assert images.shape == (B, H, W, C)

    train_s, mean, std = _host_sampling(images)
    Wall = _build_weights(train_s, mean, std)        # [B,6,100]
    pos = _pos_features()                            # [N_PIX,2]

    flat = images.reshape(B, N_PIX, C)
    feats = np.empty((B, 6, N_PIX), np.float32)
    feats[:, 0:3, :] = flat.transpose(0, 2, 1)
    feats[:, 3:5, :] = pos.T[None]
    feats[:, 5, :] = 1.0
    f16h = feats.astype(np.float16)
    f16l = (feats - f16h.astype(np.float32)).astype(np.float16)
    w16h = Wall.astype(np.float16)
    w16l = (Wall - w16h.astype(np.float32)).astype(np.float16)
    # K=24 stacking: feats rows [fh; fl; fh; fl], W rows [Wh; Wh; Wl; Wl]
    f24 = np.concatenate([f16h, f16l, f16h, f16l], axis=1)          # [B,24,NPIX]
    w24 = np.concatenate([w16h, w16h, w16l, w16l], axis=1)          # [B,24,100]
    # pixel-major tiles: imgpm[b, p, t*3+c] = img[b, t*128+p, c]
    imgpm = np.ascontiguousarray(
        flat.reshape(B, NT, 128, C).transpose(0, 2, 1, 3)
    ).reshape(B, 128, NT * C)

    in_maps = []
    for c in range(N_CORES):
        sl = slice(c * IPC, (c + 1) * IPC)
        in_maps.append(
            {
                "feats24": np.ascontiguousarray(f24[sl]),
                "w24": np.ascontiguousarray(w24[sl].transpose(1, 0, 2)),
                "imgpm": np.ascontiguousarray(imgpm[sl]),
            }
        )
    return in_maps


def assemble_output(results: list) -> np.ndarray:
    out = np.empty((B, N_PIX, C), np.float32)
    for c in range(N_CORES):
        o = results[c]["out"]  # [IPC, 128, NT*C]
        o = o.reshape(IPC, 128, NT, C).transpose(0, 2, 1, 3).reshape(IPC, N_PIX, C)
        out[c * IPC : (c + 1) * IPC] = o
    return out.reshape(B, H, W, C)


def kernel(images: np.ndarray) -> np.ndarray:
    from concourse.bass_utils import run_bass_kernel_spmd

    in_maps = prepare_in_maps(images)
    nc = _get_nc()
    res = run_bass_kernel_spmd(nc, in_maps, core_ids=list(range(N_CORES)))
    return assemble_output(res.results)
